# revision 18
# baseline (speedup 1.0000x reference)
"""Multi-head attention (B=2, L=2048, H=1024, NH=16) on 8 TRN2 NeuronCores.

Sharding: data-parallel over batch (2) x tensor-parallel over heads (4 groups
of 4 heads).  core = b*4 + g handles batch b, heads [4g, 4g+4).  Wq/Wk/Wv are
split column-wise, Wo row-wise; each core produces a partial [L, H] output
(fp16) that the host sums per batch.

Device math (per core), fp16 matmul inputs / fp32 PSUM accumulation:
  QT = (Wq*0.125)^T x^T          [256, 2048]  (softmax scale folded into Wq)
  KT = Wk^T y^T                  [256, 2048]
  V  = y Wv                      [2048, 256]  lk-partition layout
  heads processed in PAIRS (2p, 2p+1); per lq chunk of 1024 the 16 lk tiles
  pipeline as: exp(S(k)) x2 on ScalarE (the pacing engine) | S(k+1) pair
  row-tiled on the PE | O pairs for exps of k-1, STAGGERED one iteration so
  each col-tiled O pair [B(prev exp), A(cur exp)] alternates h-groups and
  both operands are ready -> the PE pairs them concurrently | fp16
  denominator accumulation on the DVE via scalar_tensor_tensor (4x mode)
  | hook work (projections / V / out-proj) rides in the remaining PE slots.
  Softmax denominators: acc(0..14) partition-reduced by a 64-wide ones
  matmul (col-tiled pair) + a second ones-matmul group on pt(15).
  out[lq, 1024] = O'^T_cat^T Wo  partial, stored fp16 (host sums groups).

Startup: input DMAs split across the two HWDGE queues (sync: weights+y,
scalar: x) in first-exp-gate priority order; the PE ramps its p-state on a
short junk-matmul warmup, then K/Q projections chase the DMA stream so the
first exp fires as soon as x chunk 0 lands.

PSUM: 3x[128,1024]f32 ring (S pairs, denominator reduces, proj/V/out-proj
pieces; <=3 allocs per iteration so ring WAR stays ~1 iteration back)
+ 1x[128,1024]f32 (O-pair accumulator) = 8 banks exactly.
"""

import numpy as np

B, L, H, NH, D = 2, 2048, 1024, 16, 64
GP = 4            # head-groups (tensor-parallel factor)
CH = H // GP      # 256 local projection cols per core
NP = 2            # head pairs per core
LQ = 1024         # lq chunk size
NLQ = L // LQ
NKT = L // 128    # 16 lk tiles
F16 = np.float16

_CACHE = {}


def _build():
    import concourse.mybir as mybir
    import concourse.tile as tile
    from concourse import bacc

    dt = mybir.dt
    f32, fp16 = dt.float32, dt.float16
    Exp = mybir.ActivationFunctionType.Exp
    MUL, ADD = mybir.AluOpType.mult, mybir.AluOpType.add

    nc = bacc.Bacc("TRN2", target_bir_lowering=False, debug=False)
    # inputs host-packed partition-major so each DMA is 128 contiguous runs
    xT = nc.declare_dram_parameter("xT", [128, NLQ, 2, 8, 512], fp16,
                                   isOutput=False)
    yT = nc.declare_dram_parameter("yT", [128, NLQ, 2, 8, 512], fp16,
                                   isOutput=False)
    wq = nc.declare_dram_parameter("wq", [128, 2, 8, 128], fp16,
                                   isOutput=False)
    wk = nc.declare_dram_parameter("wk", [128, 2, 8, 128], fp16,
                                   isOutput=False)
    wv = nc.declare_dram_parameter("wv", [128, 8, CH], fp16, isOutput=False)
    wo = nc.declare_dram_parameter("wo", [128, 2, H], fp16, isOutput=False)
    out = nc.declare_dram_parameter("out", [L, H], fp16, isOutput=True)

    with tile.TileContext(nc) as tc:
        with (
            tc.tile_pool(name="w", bufs=1) as wpool,
            tc.tile_pool(name="acts", bufs=1) as apool,
            tc.tile_pool(name="psA", bufs=3, space="PSUM") as psA,
            tc.tile_pool(name="psO", bufs=1, space="PSUM") as psO,
            tc.tile_pool(name="pt", bufs=12) as ptpool,
            tc.tile_pool(name="accp", bufs=4) as accpool,
            tc.tile_pool(name="oT", bufs=1) as otpool,
            tc.tile_pool(name="sm", bufs=2) as smpool,
            tc.tile_pool(name="osb", bufs=4) as opool,
        ):
            ones64 = wpool.tile([128, 64], fp16, tag="ones64")
            nc.vector.memset(ones64, 1.0)
            warm = wpool.tile([128, 512], fp16, tag="warm")
            nc.vector.memset(warm, 0.0)

            # ---- input DMAs.  First-exp gate (wk ct0, wq ct0, y rows 0:512,
            # x rows 0:2048 of chunk 0) leads on BOTH HWDGE queues; the rest
            # follows in ride-deadline order on the sync queue ---------------
            wk_sb = wpool.tile([128, 2, 8, 128], fp16, tag="wk")
            wq_sb = wpool.tile([128, 2, 8, 128], fp16, tag="wq")
            yT_sb = apool.tile([128, NLQ, 2, 8, 512], fp16, tag="yT")
            xT_sb = apool.tile([128, NLQ, 2, 8, 512], fp16, tag="xT")
            wv_sb = wpool.tile([128, 8, CH], fp16, tag="wv")
            wo_sb = wpool.tile([128, 2, H], fp16, tag="wo")

            nc.sync.dma_start(wk_sb[:, 0], wk[:, 0])
            nc.sync.dma_start(yT_sb[:, 0, 0], yT[:, 0, 0])
            nc.sync.dma_start(xT_sb[:, 0, 0], xT[:, 0, 0])
            nc.sync.dma_start(wq_sb[:, 0], wq[:, 0])
            nc.sync.dma_start(xT_sb[:, 0, 1], xT[:, 0, 1])
            nc.sync.dma_start(wv_sb, wv[:, :, :])
            nc.sync.dma_start(yT_sb[:, 0, 1], yT[:, 0, 1])
            nc.sync.dma_start(wk_sb[:, 1], wk[:, 1])
            nc.sync.dma_start(wq_sb[:, 1], wq[:, 1])
            nc.sync.dma_start(yT_sb[:, 1, 0], yT[:, 1, 0])
            nc.sync.dma_start(yT_sb[:, 1, 1], yT[:, 1, 1])
            nc.sync.dma_start(xT_sb[:, 1, 0], xT[:, 1, 0])
            nc.sync.dma_start(xT_sb[:, 1, 1], xT[:, 1, 1])
            nc.sync.dma_start(wo_sb, wo[:, :, :])

            # prefetch the exp activation table while input DMAs run (after
            # the scalar-queue dma_starts so it doesn't delay the gate DMAs)
            dummy = smpool.tile([1, 8], f32, tag="dummy")
            nc.vector.memset(dummy, 0.0)
            nc.scalar.activation(dummy, dummy, Exp)

            # HAM warm-up: keep the PE busy through the DMA window so the
            # first projections run at 2.4 GHz, not the cold 1.2
            wps = psA.tile([128, LQ], f32, tag="psA", name="warmps")
            for _ in range(10):
                nc.tensor.matmul(wps[0:64, 0:512], lhsT=ones64, rhs=warm,
                                 start=True, stop=True)

            qT_sb = apool.tile([128, 2, L], fp16, tag="qT")
            kT_sb = apool.tile([128, 2, L], fp16, tag="kT")
            v_sb = apool.tile([128, NKT, CH], fp16, tag="v")

            def proj_group(w_sb, act_sb, dst, ct, lh, sl):
                # dst[:, ct, lh*LQ+sl*512 : +512] via one 8-matmul psum group
                ps = psA.tile([128, LQ], f32, tag="psA", name="projps")
                off = lh * LQ + sl * 512
                for ht in range(8):
                    nc.tensor.matmul(
                        ps[:, 0:512],
                        lhsT=w_sb[:, ct, ht, :],
                        rhs=act_sb[:, lh, sl, ht, :],
                        start=(ht == 0), stop=(ht == 7),
                    )
                nc.vector.tensor_copy(dst[:, ct, off:off + 512], ps[:, 0:512])

            def vpair(j0):
                # V tiles (j0, j0+1), 16 matmuls + one [128,512] copy sharing
                # one psum alloc; split into two 8-matmul thunks
                cell = {}

                def half(r):
                    def thunk():
                        if r == 0:
                            cell["ps"] = psA.tile([128, LQ], f32, tag="psA",
                                                  name=f"psv{j0}")
                        psv = cell["ps"]
                        lkt = j0 + r
                        for ht in range(8):
                            nc.tensor.matmul(
                                psv[:, r * CH:(r + 1) * CH],
                                lhsT=yT_sb[:, lkt // 8, (lkt % 8) // 4, ht,
                                           (lkt % 4) * 128:
                                           (lkt % 4 + 1) * 128],
                                rhs=wv_sb[:, ht, :],
                                start=(ht == 0), stop=(ht == 7),
                            )
                        if r == 1:
                            nc.vector.tensor_copy(
                                v_sb[:, j0:j0 + 2, :].rearrange(
                                    "p a b -> p (a b)"),
                                psv[:, 0:2 * CH])
                    return thunk
                return half(0), half(1)

            def s3_piece(ci, oT_sb, mt, pool=None, act_copy=False,
                         kts=(0, 1), dst=None):
                # dst[mt*128 : +128, :] = oT[:, kts]^T @ Wo[kts], fp16
                pool = pool if pool is not None else psA
                pso = pool.tile([128, LQ], f32,
                                tag="psA" if pool is psA else "psO",
                                name="s3pso")
                for nt in range(2):
                    for i, kt in enumerate(kts):
                        nc.tensor.matmul(
                            pso[:, nt * 512:(nt + 1) * 512],
                            lhsT=oT_sb[:, kt, mt * 128:(mt + 1) * 128],
                            rhs=wo_sb[:, kt, nt * 512:(nt + 1) * 512],
                            start=(i == 0), stop=(i == len(kts) - 1),
                        )
                osb = opool.tile([128, LQ], fp16, tag="osb")
                if act_copy:
                    nc.scalar.copy(osb, pso)
                else:
                    nc.vector.tensor_copy(osb, pso)
                if dst is None:
                    dst = out[ci * LQ + mt * 128:ci * LQ + (mt + 1) * 128, :]
                nc.sync.dma_start(dst, osb)

            def emit_S_pair(p, ci, lkt):
                # two K=64 matmuls per sl, row-tiled (partitions 0-63 vs
                # 64-127) so each adjacent pair runs concurrently on the PE
                psS = [psA.tile([128, LQ], f32, tag="psA", name=f"psS{h}")
                       for h in range(2)]
                for sl in range(2):
                    for h in range(2):
                        po = slice(64 * h, 64 * h + 64)
                        nc.tensor.matmul(
                            psS[h][:, sl * 512:(sl + 1) * 512],
                            lhsT=kT_sb[po, p, lkt * 128:(lkt + 1) * 128],
                            rhs=qT_sb[po, p,
                                      ci * LQ + sl * 512:
                                      ci * LQ + (sl + 1) * 512],
                            start=True, stop=True,
                        )
                return psS

            pipe = {}

            def s2_pair(p, ci, oT_sb, extra=None, nxt=None, heavy=False,
                        first=False):
                # one head pair x one lq chunk: 16 lkt periods of
                # exp x2 | O pairs for exps of k-1 | hooks | S(k+1) h0 then
                # h1.  O matmuls are staggered one iteration so every
                # col-tiled pair [B(prev exp), A(cur exp)] alternates
                # h-groups with both pt tiles ready -> true PE concurrency.
                # psA ring discipline: the S pair is allocated at the top of
                # each iteration and hooks contribute AT MOST one further
                # alloc (emitted last in the hook list) -> S(k+1)h0 WARs at
                # worst exp(k,h0) and S(k+1)h1 at worst exp(k,h1); h1 is
                # emitted last so that wait never head-of-line blocks.
                psO_c = psO.tile([128, LQ], f32, tag="psO", name="psOc")
                # col-tiled pair shares banks: zero, then accumulate with
                # start=False throughout.  After the first chunk the zeroing
                # rides on the PE as two full-region zero-weight matmuls
                # (start=True) so it never queues behind the previous
                # chunk's normalize on the DVE.
                if first:
                    nc.vector.memset(psO_c, 0.0)

                def ps_clear(t):
                    for c2 in (0, 512):
                        nc.tensor.matmul(t[:, c2:c2 + 512],
                                         lhsT=warm[:, 0:128], rhs=warm,
                                         start=True, stop=True)
                acc = [accpool.tile([128, LQ], fp16, tag="acc",
                                    name=f"acc{h}") for h in range(2)]
                psS = pipe.pop("psS", None)
                ptq = {}
                sums_cell = {}

                def emit_O(lkt, sl, h):
                    nc.tensor.matmul(
                        psO_c[h * 64:(h + 1) * 64,
                              sl * 512:(sl + 1) * 512],
                        lhsT=v_sb[:, lkt,
                                  p * 128 + h * 64:p * 128 + (h + 1) * 64],
                        rhs=ptq[lkt][h][:, sl * 512:(sl + 1) * 512],
                        start=False,
                        stop=(lkt == NKT - 1 and h == 1),
                        skip_group_check=True,
                    )

                def emit_S_h(psS_t, sp, sci, lkt, h):
                    po = slice(64 * h, 64 * h + 64)
                    for sl in range(2):
                        nc.tensor.matmul(
                            psS_t[h][:, sl * 512:(sl + 1) * 512],
                            lhsT=kT_sb[po, sp, lkt * 128:(lkt + 1) * 128],
                            rhs=qT_sb[po, sp,
                                      sci * LQ + sl * 512:
                                      sci * LQ + (sl + 1) * 512],
                            start=True, stop=True,
                        )

                def sums_mms(rhs_of, stop_grp):
                    # ones64^T @ rhs -> [64, lq] replicated, col-tiled into
                    # one psA tile (h0 rows 0-63, h1 64-127)
                    sums = sums_cell["t"]
                    for hs in range(2):
                        for h in range(2):
                            nc.tensor.matmul(
                                sums[h * 64:(h + 1) * 64,
                                     hs * 512:(hs + 1) * 512],
                                lhsT=ones64,
                                rhs=rhs_of(h)[:, hs * 512:(hs + 1) * 512],
                                start=False,
                                stop=(stop_grp and h == 1),
                                skip_group_check=True,
                            )

                for lkt in range(NKT):
                    # top-of-iteration allocations fix the psA ring phase
                    if lkt + 1 < NKT:
                        psS_n = [psA.tile([128, LQ], f32, tag="psA",
                                          name=f"psS{h}") for h in range(2)]
                        s_args = (p, ci, lkt + 1)
                    elif nxt is not None:
                        psS_n = [psA.tile([128, LQ], f32, tag="psA",
                                          name=f"psS{h}") for h in range(2)]
                        s_args = (nxt[0], nxt[1], 0)
                    else:
                        psS_n = None
                    pt = [ptpool.tile([128, LQ], fp16, tag="pt",
                                      name=f"pt{h}") for h in range(2)]
                    ptq[lkt] = pt
                    for h in range(2):
                        nc.scalar.activation(pt[h], psS[h], Exp)
                    # DVE: denominator accumulation (and sums memset rides
                    # on the idle GpSimd engine)
                    if lkt == NKT - 2:
                        sums_cell["t"] = psA.tile([128, LQ], f32,
                                                  tag="psA", name="sums")
                        ps_clear(sums_cell["t"])
                    if lkt < NKT - 1:
                        for h in range(2):
                            if lkt == 0:
                                nc.vector.tensor_copy(acc[h], pt[h])
                            else:
                                nc.vector.tensor_add(acc[h], acc[h], pt[h])
                    # PE: O pairs for exps of lkt-1 (carry B(h1) from lkt-2)
                    if lkt >= 1:
                        if lkt >= 2:
                            emit_O(lkt - 2, 1, 1)
                        emit_O(lkt - 1, 0, 0)
                        emit_O(lkt - 1, 1, 0)
                        emit_O(lkt - 1, 0, 1)
                        if lkt >= 2:
                            del ptq[lkt - 2]
                    # PE: hooks and the next S tiles.  Hook-heavy chunks
                    # interleave (first thunk, S h0, remaining thunks, S h1
                    # last) so the S WAR waits never head-of-line block;
                    # light chunks emit the S halves adjacently right after
                    # the O pairs so they run as a concurrent row-tiled pair.
                    hooks = list(extra.get(lkt, ())) if extra else []
                    if heavy:
                        if hooks:
                            hooks[0]()
                        if psS_n is not None:
                            emit_S_h(psS_n, s_args[0], s_args[1],
                                     s_args[2], 0)
                        for job in hooks[1:]:
                            job()
                        if psS_n is not None:
                            emit_S_h(psS_n, s_args[0], s_args[1],
                                     s_args[2], 1)
                    else:
                        if psS_n is not None:
                            emit_S_h(psS_n, s_args[0], s_args[1],
                                     s_args[2], 0)
                            emit_S_h(psS_n, s_args[0], s_args[1],
                                     s_args[2], 1)
                        for job in hooks:
                            job()
                    if psS_n is not None and lkt == NKT - 1:
                        pipe["psS"] = psS_n
                    if lkt == 0 and not first:
                        ps_clear(psO_c)
                    if lkt == NKT - 1:
                        # partial denominator reduce of acc(0..14)
                        sums_mms(lambda h: acc[h], False)
                    if psS_n is not None:
                        psS = psS_n
                # chunk-end leftovers: [B(14,h1) A(15,h0)] pair, the pt(15)
                # denominator reduce, [B(15,h0) A(15,h1)] pair, B(15,h1)
                emit_O(NKT - 2, 1, 1)
                emit_O(NKT - 1, 0, 0)
                sums_mms(lambda h: ptq[NKT - 1][h], True)
                emit_O(NKT - 1, 1, 0)
                emit_O(NKT - 1, 0, 1)
                emit_O(NKT - 1, 1, 1)
                sums = sums_cell["t"]
                rcp = smpool.tile([128, LQ], f32, tag="rcp")
                for hs in range(2):
                    c = slice(hs * 512, (hs + 1) * 512)
                    nc.vector.reciprocal_approx_fast(rcp[:, c], sums[:, c])
                    nc.vector.tensor_mul(
                        oT_sb[:, p, c], psO_c[:, c], rcp[:, c])

            # ---- emission order ------------------------------------------
            oT = [otpool.tile([128, 2, LQ], fp16, tag="oT", name=f"oT{i}")
                  for i in range(NLQ)]
            # startup: the first-exp gate (K ct0 lh0, Q ct0 lh0, S(0) pair)
            # plus V(0,1), all riding the input-DMA window; K001 fills the
            # PE gap between yT00 and xT00 arrival
            def junk(n):
                for _ in range(n):
                    nc.tensor.matmul(wps[0:64, 0:512], lhsT=ones64,
                                     rhs=warm, start=True, stop=True)

            proj_group(wk_sb, yT_sb, kT_sb, 0, 0, 0)
            junk(4)
            proj_group(wq_sb, xT_sb, qT_sb, 0, 0, 0)
            junk(4)
            proj_group(wq_sb, xT_sb, qT_sb, 0, 0, 1)
            pipe["psS"] = emit_S_pair(0, 0, 0)
            v01a, v01b = vpair(0)
            v01a()
            v01b()

            def make_hook(sched):
                def hook(lkt):
                    for job in sched.get(lkt, ()):
                        job()
                return hook

            def pj2(w_sb, act_sb, dst, ct, lh, sl):
                # one projection psum group split into two 4-matmul thunks
                cell = {}

                def half(r):
                    def thunk():
                        if r == 0:
                            cell["ps"] = psA.tile(
                                [128, LQ], f32, tag="psA",
                                name=f"pjps{ct}_{lh}_{sl}_{id(w_sb) % 97}")
                        ps = cell["ps"]
                        for ht in range(4 * r, 4 * r + 4):
                            nc.tensor.matmul(
                                ps[:, 0:512],
                                lhsT=w_sb[:, ct, ht, :],
                                rhs=act_sb[:, lh, sl, ht, :],
                                start=(ht == 0), stop=(ht == 7),
                            )
                        if r == 1:
                            nc.vector.tensor_copy(
                                dst[:, ct, lh * LQ + sl * 512:
                                    lh * LQ + (sl + 1) * 512], ps[:, 0:512])
                    return thunk
                return half(0), half(1)

            def sched_projs(sched, groups, slot_pairs):
                for g, (sa, sb_) in zip(groups, slot_pairs):
                    a, b = pj2(*g)
                    sched.setdefault(sa, []).append(a)
                    sched.setdefault(sb_, []).append(b)

            # Hook schedules.  Rules: at most ONE psA-allocating thunk per
            # iteration (pj r0 / vpair a / s3), listed LAST in its iteration
            # so its WAR wait never head-of-line blocks other hook work;
            # non-allocating halves (r1 / b) go first.  Iters 14/15 stay
            # clear (the sums alloc is iteration 14's alloc).
            K001 = pj2(wk_sb, yT_sb, kT_sb, 0, 0, 1)
            K010 = pj2(wk_sb, yT_sb, kT_sb, 0, 1, 0)
            K011 = pj2(wk_sb, yT_sb, kT_sb, 0, 1, 1)
            K100 = pj2(wk_sb, yT_sb, kT_sb, 1, 0, 0)
            K101 = pj2(wk_sb, yT_sb, kT_sb, 1, 0, 1)
            K110 = pj2(wk_sb, yT_sb, kT_sb, 1, 1, 0)
            K111 = pj2(wk_sb, yT_sb, kT_sb, 1, 1, 1)
            Q100 = pj2(wq_sb, xT_sb, qT_sb, 1, 0, 0)
            Q101 = pj2(wq_sb, xT_sb, qT_sb, 1, 0, 1)
            Q010 = pj2(wq_sb, xT_sb, qT_sb, 0, 1, 0)
            Q011 = pj2(wq_sb, xT_sb, qT_sb, 0, 1, 1)
            Q110 = pj2(wq_sb, xT_sb, qT_sb, 1, 1, 0)
            Q111 = pj2(wq_sb, xT_sb, qT_sb, 1, 1, 1)
            V = {j: vpair(2 * j) for j in range(1, 8)}
            h0_sched = {
                0: [V[1][0]],
                1: [V[1][1], K001[0]],
                2: [K001[1], V[2][0]],
                3: [V[2][1], K010[0]],
                4: [K010[1], V[3][0]],
                5: [V[3][1], K011[0]],
                6: [K011[1], V[4][0]],
                7: [V[4][1], K100[0]],
                8: [K100[1], V[5][0]],
                9: [V[5][1], Q100[0]],
                10: [Q100[1], V[6][0]],
                11: [V[6][1], Q101[0]],
                12: [Q101[1], V[7][0]],
                13: [V[7][1]],
            }
            h1_sched = {
                0: [K101[0]],
                1: [K101[1], K110[0]],
                2: [K110[1], K111[0]],
                3: [K111[1], Q110[0]],
                4: [Q110[1], Q111[0]],
                5: [Q111[1]],
            }
            h2_sched = {
                0: [Q010[0]],
                1: [Q010[1], Q011[0]],
                2: [Q011[1]],
            }
            for i, mt in enumerate(range(8)):
                h2_sched.setdefault(4 + i, []).append(
                    lambda mt=mt: s3_piece(0, oT[0], mt))

            # chunk order p0c0, p1c0, p1c1, p0c1: the tail s3(c1) pieces
            # then have their pair-1 operand (oT[1][:,1]) a full chunk early,
            # so emitting kt=1 matmuls first lets them pre-run through the
            # PE wait queue while the last chunk still streams exps
            s2_pair(0, 0, oT[0], extra=h0_sched, nxt=(1, 0), heavy=True,
                    first=True)
            s2_pair(1, 0, oT[0], extra=h1_sched, nxt=(1, 1))
            s2_pair(1, 1, oT[1], extra=h2_sched, nxt=(0, 1))
            s2_pair(0, 1, oT[1])
            # tail: out-projection of chunk 1 (psA ring is free now; psO
            # frees after the last normalize); alternate the psum->sbuf
            # casts between DVE and ScalarE so they pipeline
            for mt in range(8):
                s3_piece(1, oT[1], mt,
                         pool=(psO if mt % 3 == 2 else psA),
                         act_copy=bool(mt % 2), kts=(1, 0))
    nc.compile()
    return nc


def _get_nc():
    if "nc" not in _CACHE:
        _CACHE["nc"] = _build()
    return _CACHE["nc"]


def _pack_pm(a, t):
    # [t*128, N] -> [128, t, N] partition-major
    return a.reshape(t, 128, -1).transpose(1, 0, 2)


def _pack_act(a):
    # x[b] [L, H] -> xT packed [128, NLQ(lh), 2(sl), 8(t), 512] fp16
    v = _pack_pm(np.ascontiguousarray(a.T), 8)          # [128, 8, L]
    v = v.reshape(128, 8, NLQ, 2, 512).transpose(0, 2, 3, 1, 4)
    return np.ascontiguousarray(v).astype(F16)


def _pack_w(a):
    # W-shard [H, CH] -> [128, 2(ct), 8(t), 128] fp16
    v = _pack_pm(a, 8)                                  # [128, 8, CH]
    v = v.reshape(128, 8, 2, 128).transpose(0, 2, 1, 3)
    return np.ascontiguousarray(v).astype(F16)


def _in_maps(x, y, Wq, Wk, Wv, Wo):
    maps = []
    for core in range(8):
        b, g = core // GP, core % GP
        cs = slice(g * CH, (g + 1) * CH)
        maps.append({
            "xT": _pack_act(x[b]),
            "yT": _pack_act(y[b]),
            "wq": _pack_w(Wq[:, cs] * np.float32(0.125)),
            "wk": _pack_w(Wk[:, cs]),
            "wv": np.ascontiguousarray(_pack_pm(Wv[:, cs], 8)).astype(F16),
            "wo": np.ascontiguousarray(_pack_pm(Wo[cs, :], 2)).astype(F16),
        })
    return maps


def _install_ntff_hook():
    """Provide the antenv.axon_hooks shim missing from this container so
    run_bass_kernel_spmd(trace=True) can drive NTFF profiling via ctypes."""
    import sys
    import types
    try:
        from antenv.axon_hooks import get_axon_ntff_profile_hook  # noqa: F401
        return
    except ImportError:
        pass
    from trn_agent_boot.trn_boot import _ntff_profile_via_ctypes
    hook = _ntff_profile_via_ctypes("/opt/axon/libaxon_pjrt.so")
    mod = types.ModuleType("antenv.axon_hooks")
    mod.get_axon_ntff_profile_hook = lambda: hook
    mod.set_axon_ntff_profile_hook = lambda h: None
    sys.modules["antenv.axon_hooks"] = mod


def _run(inputs, trace=False):
    from concourse import bass_utils

    if trace:
        _install_ntff_hook()

    x, y, bias = inputs["x"], inputs["y"], inputs["bias"]
    if np.count_nonzero(np.asarray(bias)):
        raise NotImplementedError("nonzero attention bias not supported")
    nc = _get_nc()
    maps = _in_maps(np.asarray(x, np.float32), np.asarray(y, np.float32),
                    np.asarray(inputs["Wq"], np.float32),
                    np.asarray(inputs["Wk"], np.float32),
                    np.asarray(inputs["Wv"], np.float32),
                    np.asarray(inputs["Wo"], np.float32))
    res = bass_utils.run_bass_kernel_spmd(
        nc, maps, list(range(8)), trace=trace)
    out = np.zeros((B, L, H), np.float32)
    for core in range(8):
        out[core // GP] += res.results[core]["out"].astype(np.float32)
    return out, res


def kernel(**inputs):
    out, _ = _run(inputs, trace=False)
    return out
